# revision 5
# baseline (speedup 1.0000x reference)
"""Trainium2 Bass kernel for CrossAttentionWithDistBias.

Sharding: rows of (B=2, Tr=2048) split across 8 cores -> each core owns
one batch (c//4) and a 512-row block ((c%4)*512), computes all heads for
its rows (RBF bias work is shared across heads, so head-sharding would
duplicate the dominant exp cost 8x), and writes its slice of out/attn.
No collectives needed.

Device pipeline per core:
  D2^T via K=5 feature matmul; d = ACT sqrt; z[(l8,n),r] via selection
  matmul (puts the RBF index n on partitions); E = ACT exp (per-partition
  c0 bias); per-head bias via block-diagonal matmul; PE-transpose to
  [r,(l,h)]; QK matmul with K=33 (mask row folded in) + identity-matmul
  bias add into the same PSUM; ACT exp with fused row-sum (softmax denom);
  unnormalized attn @ v with the 1/rowsum folded into the small output;
  gpsimd normalizes attn for the HBM write.
"""

import numpy as np
import ml_dtypes

import concourse.bacc as bacc
import concourse.bass as bass
import concourse.mybir as mybir
import concourse.tile as tile
from concourse.bass_utils import run_bass_kernel_spmd

F32 = mybir.dt.float32
BF16 = mybir.dt.bfloat16
AF = mybir.ActivationFunctionType
ALU = mybir.AluOpType

B, TR, TL, D, H, DH, NR = 2, 2048, 1024, 256, 8, 32, 16
RCORE = 512
SCALE = DH ** -0.5
BIAS_SCALE = 2.0
MASKNEG = -30000.0
NCORES = 8

_CACHE: dict = {}


def _build_module():
    nc = bacc.Bacc("TRN2", target_bir_lowering=False, debug=False,
                   enable_asserts=False, num_devices=NCORES)

    def din(name, p, f, dt=F32):
        return nc.dram_tensor(name, [p, f], dt, kind="ExternalInput").ap()

    ins = dict(
        recT=din("recT", 128, 2 * RCORE),
        ligT=din("ligT", 128, 2 * TL),
        WqP=din("WqP", 128, 2 * 512),
        WkP=din("WkP", 128, 2 * 512),
        Wv=din("Wv", 128, 2 * 256),
        Wo=din("Wo", 128, 2 * 256),
        bqP=din("bqP", 128, 4),
        bkP=din("bkP", 128, 4),
        bvB=din("bvB", 128, 256),
        boS=din("boS", 128, 2),
        featL=din("featL", 5, TL),
        featR=din("featR", 5, RCORE),
        Wbd=din("Wbd", 128, 64),
        selz=din("selz", 128, 8 * 128),
        c0col=din("c0col", 128, 1),
        maskrow=din("maskrow", 1, TL),
        I128f=din("I128f", 128, 128),
        repM=din("repM", 8, 256),
        I128h=din("I128h", 128, 128, BF16),
    )
    ins["attn_o"] = nc.dram_tensor("attn_o", [H, RCORE, TL], F32,
                                   kind="ExternalOutput").ap()
    ins["out_o"] = nc.dram_tensor("out_o", [RCORE, D], F32,
                                  kind="ExternalOutput").ap()

    with tile.TileContext(nc) as tc:
        _body(nc, tc, ins)
    nc.compile()
    return nc


def _body(nc, tc, t):
    from contextlib import ExitStack
    es = ExitStack()
    with es:
        P = es.enter_context(tc.tile_pool(name="persist", bufs=1))
        recT_s = P.tile([128, 2 * RCORE], F32, tag="recT")
        ligT_s = P.tile([128, 2 * TL], F32, tag="ligT")
        WqP_s = P.tile([128, 2 * 512], F32, tag="WqP")
        WkP_s = P.tile([128, 2 * 512], F32, tag="WkP")
        Wv_s = P.tile([128, 2 * 256], F32, tag="Wv")
        Wo_s = P.tile([128, 2 * 256], F32, tag="Wo")
        bqP_s = P.tile([128, 4], F32, tag="bqP")
        bkP_s = P.tile([128, 4], F32, tag="bkP")
        bvB_s = P.tile([128, 256], F32, tag="bvB")
        boS_s = P.tile([128, 2], F32, tag="boS")
        featL_s = P.tile([5, TL], F32, tag="featL")
        featR_s = P.tile([5, RCORE], F32, tag="featR")
        Wbd_s = P.tile([128, 64], F32, tag="Wbd")
        selz_s = P.tile([128, 8 * 128], F32, tag="selz")
        c0_s = P.tile([128, 1], F32, tag="c0col")
        mrow_s = P.tile([1, TL], F32, tag="mask")
        I128f_s = P.tile([128, 128], F32, tag="idf")
        repM_s = P.tile([8, 256], F32, tag="repM")
        I128h_s = P.tile([128, 128], BF16, tag="idh")
        qTx_s = [P.tile([128, RCORE], F32, tag=f"qTx{i}", name=f"qTx{i}")
                 for i in range(4)]
        kTx_s = [P.tile([128, TL], F32, tag=f"kTx{i}", name=f"kTx{i}")
                 for i in range(4)]
        v_s = [P.tile([128, 256], F32, tag=f"v{i}", name=f"v{i}")
               for i in range(8)]
        rhsA_s = P.tile([128, 16 * RCORE], F32, tag="rhsA")
        brt_s = P.tile([128, 2 * 64 * 128], BF16, tag="brt")

        for name, dst in [("recT", recT_s), ("ligT", ligT_s), ("WqP", WqP_s),
                          ("WkP", WkP_s), ("Wv", Wv_s), ("Wo", Wo_s),
                          ("bqP", bqP_s), ("bkP", bkP_s), ("bvB", bvB_s),
                          ("boS", boS_s), ("featL", featL_s),
                          ("featR", featR_s), ("Wbd", Wbd_s),
                          ("selz", selz_s), ("c0col", c0_s),
                          ("maskrow", mrow_s), ("I128f", I128f_s),
                          ("repM", repM_s),
                          ("I128h", I128h_s)]:
            nc.sync.dma_start(dst[:], t[name])

        # ---------- stage A ----------
        with tc.tile_pool(name="psA", bufs=2, space="PSUM") as psA:
            for tt in range(4):
                ps = psA.tile([128, 1024], F32, tag="ps")
                for kt in range(2):
                    nc.tensor.matmul(
                        ps[:, 0:RCORE],
                        WqP_s[:, kt * 512 + tt * 128: kt * 512 + (tt + 1) * 128],
                        recT_s[:, kt * RCORE:(kt + 1) * RCORE],
                        start=(kt == 0), stop=(kt == 1))
                nc.scalar.activation(qTx_s[tt][:], ps[:, 0:RCORE],
                                     AF.Identity, bias=bqP_s[:, tt:tt + 1])
                nc.gpsimd.memset(qTx_s[tt][32:33, :], 1.0)
                nc.gpsimd.memset(qTx_s[tt][96:97, :], 1.0)
            for tt in range(4):
                ps = psA.tile([128, 1024], F32, tag="ps")
                for kt in range(2):
                    for nh in range(2):
                        nc.tensor.matmul(
                            ps[:, nh * 512:(nh + 1) * 512],
                            WkP_s[:, kt * 512 + tt * 128: kt * 512 + (tt + 1) * 128],
                            ligT_s[:, kt * TL + nh * 512: kt * TL + (nh + 1) * 512],
                            start=(kt == 0), stop=(kt == 1))
                nc.scalar.activation(kTx_s[tt][:], ps[:],
                                     AF.Identity, bias=bkP_s[:, tt:tt + 1])
                nc.sync.dma_start(kTx_s[tt][32:33, :], mrow_s[:])
                nc.sync.dma_start(kTx_s[tt][96:97, :], mrow_s[:])
            for c in range(8):
                ps = psA.tile([128, 1024], F32, tag="ps")
                for kt in range(2):
                    nc.tensor.matmul(
                        ps[:, 0:256],
                        ligT_s[:, kt * TL + c * 128: kt * TL + (c + 1) * 128],
                        Wv_s[:, kt * 256:(kt + 1) * 256],
                        start=(kt == 0), stop=(kt == 1))
                nc.vector.tensor_tensor(v_s[c][:], ps[:, 0:256], bvB_s[:],
                                        op=ALU.add)
            for c in range(8):
                ps = psA.tile([128, 1024], F32, tag="ps")
                nc.tensor.matmul(ps[:, 0:RCORE],
                                 featL_s[:, c * 128:(c + 1) * 128],
                                 featR_s[:], start=True, stop=True)
                for sub in range(2):
                    w = 2 * c + sub
                    nc.vector.tensor_scalar_max(
                        rhsA_s[64:128, w * RCORE:(w + 1) * RCORE],
                        ps[sub * 64:(sub + 1) * 64, 0:RCORE], 0.0)
                    nc.scalar.activation(
                        rhsA_s[0:64, w * RCORE:(w + 1) * RCORE],
                        rhsA_s[64:128, w * RCORE:(w + 1) * RCORE], AF.Sqrt)

        brt_v = brt_s[:].rearrange("p (rs gp c) -> p rs gp c", rs=2, gp=64)
        brt_mm = brt_s[:].rearrange("p (rs gp g2 l8 h) -> p rs gp g2 l8 h",
                                    rs=2, gp=64, g2=2, l8=8, h=8)

        for half in range(2):
            r0 = half * 256
            # ---------- stage B ----------
            with tc.tile_pool(name="psZ", bufs=2, space="PSUM") as psZ, \
                 tc.tile_pool(name="psB", bufs=1, space="PSUM") as psB, \
                 tc.tile_pool(name="psT", bufs=1, space="PSUM") as psT, \
                 tc.tile_pool(name="sbB", bufs=2) as sbB:
                bt = bsb = tt8 = None
                for quad in range(32):
                    zt = psZ.tile([128, 1024], F32, tag="z")
                    for gi in range(4):
                        g = quad * 4 + gi
                        w, go = g // 8, g % 8
                        nc.tensor.matmul(
                            zt[:, gi * 256:(gi + 1) * 256],
                            selz_s[:, go * 128:(go + 1) * 128],
                            rhsA_s[:, w * RCORE + r0: w * RCORE + r0 + 256],
                            start=True, stop=True)
                    et = sbB.tile([128, 1024], F32, tag="E")
                    nc.scalar.activation(et[:], zt[:], AF.Exp, bias=c0_s[:])
                    if quad % 2 == 0:
                        bt = psB.tile([128, 1024], F32, tag="bps")
                    for gi in range(4):
                        g = quad * 4 + gi
                        g2 = g % 2
                        gpl = ((quad % 2) * 4 + gi) // 2
                        nc.tensor.matmul(
                            bt[g2 * 64:(g2 + 1) * 64, gpl * 256:(gpl + 1) * 256],
                            Wbd_s[:], et[:, gi * 256:(gi + 1) * 256],
                            start=True, stop=True)
                    if quad % 2 == 1:
                        bsb = sbB.tile([128, 1024], F32, tag="bsb")
                        nc.vector.tensor_copy(bsb[:], bt[:])
                        tt8 = psT.tile([128, 1024], F32, tag="tps")
                        for gp in range(4):
                            for rsx in range(2):
                                nc.tensor.transpose(
                                    tt8[:, (gp * 2 + rsx) * 128:(gp * 2 + rsx + 1) * 128],
                                    bsb[:, gp * 256 + rsx * 128: gp * 256 + rsx * 128 + 128],
                                    I128f_s[:])
                        gp0 = (quad // 2) * 4
                        ov = tt8[:].rearrange("p (gp rsx c) -> p gp rsx c",
                                              gp=4, rsx=2)
                        nc.vector.tensor_copy(
                            brt_v[:, :, gp0:gp0 + 4, :].rearrange(
                                "p rsx gp c -> p gp rsx c"), ov)

            # ---------- stage C ----------
            with tc.tile_pool(name="psL", bufs=2, space="PSUM") as psL, \
                 tc.tile_pool(name="psT2", bufs=1, space="PSUM") as psT2, \
                 tc.tile_pool(name="psAV", bufs=2, space="PSUM") as psAV, \
                 tc.tile_pool(name="sbC", bufs=2) as sbC, \
                 tc.tile_pool(name="sbR", bufs=1) as sbR:
                for rs in range(2):
                    cg = half * 2 + rs
                    rsum = sbR.tile([128, 8], F32, tag="rsum")
                    rsi = sbR.tile([128, 8], F32, tag="rsi")
                    av = [psAV.tile([128, 128], F32, tag="av", name=f"av{i}")
                          for i in range(2)]
                    for h in range(8):
                        tt, j = h // 2, h % 2
                        pl = psL.tile([128, 1024], F32, tag="L")
                        for nh in range(2):
                            nc.tensor.matmul(
                                pl[:, nh * 512:(nh + 1) * 512],
                                qTx_s[tt][64 * j:64 * j + 33,
                                          cg * 128:(cg + 1) * 128],
                                kTx_s[tt][64 * j:64 * j + 33,
                                          nh * 512:(nh + 1) * 512],
                                start=True, stop=False)
                            nc.tensor.matmul(
                                pl[:, nh * 512:(nh + 1) * 512],
                                I128h_s[:],
                                brt_mm[:, rs, nh * 32:(nh + 1) * 32, :, :, h],
                                start=False, stop=True)
                        aun = sbC.tile([128, 1024], F32, tag="aun")
                        nc.scalar.activation(aun[:], pl[:], AF.Exp,
                                             accum_out=rsum[:, h:h + 1])
                        nc.vector.reciprocal(rsi[:, h:h + 1], rsum[:, h:h + 1])
                        t2 = psT2.tile([128, 1024], F32, tag="t2")
                        for lt in range(8):
                            nc.tensor.transpose(
                                t2[:, lt * 128:(lt + 1) * 128],
                                aun[:, lt * 128:(lt + 1) * 128], I128f_s[:])
                        atT = sbC.tile([128, 1024], F32, tag="atT")
                        nc.vector.tensor_copy(atT[:], t2[:])
                        for lt in range(8):
                            nc.tensor.matmul(
                                av[h // 4][32 * (h % 4):32 * (h % 4) + 32, :],
                                v_s[lt][:, h * 32:(h + 1) * 32],
                                atT[:, lt * 128:(lt + 1) * 128],
                                start=(lt == 0), stop=(lt == 7),
                                tile_position=(0, 32 * (h % 4)))
                        anrm = sbC.tile([128, 1024], F32, tag="anrm")
                        nc.gpsimd.tensor_scalar_mul(anrm[:], aun[:],
                                                    rsi[:, h:h + 1])
                        nc.sync.dma_start(
                            t["attn_o"][h, cg * 128:(cg + 1) * 128, :], anrm[:])
                    # out path
                    t2 = psT2.tile([128, 1024], F32, tag="t2")
                    nc.tensor.transpose(t2[0:8, 0:128], rsi[:], I128f_s[:])
                    rsiT = sbR.tile([8, 128], F32, tag="rsiT")
                    nc.vector.tensor_copy(rsiT[:], t2[0:8, 0:128])
                    rbps = psT2.tile([128, 1024], F32, tag="t2", name="rbps")
                    for i in range(2):
                        nc.tensor.matmul(rbps[:, i * 128:(i + 1) * 128],
                                         repM_s[:, i * 128:(i + 1) * 128],
                                         rsiT[:], start=True, stop=True)
                    rbsb = sbR.tile([128, 256], F32, tag="rbsb")
                    nc.vector.tensor_copy(rbsb[:], rbps[:, 0:256])
                    oT = sbR.tile([128, 256], F32, tag="oT")
                    for i in range(2):
                        nc.vector.tensor_tensor(oT[:, i * 128:(i + 1) * 128],
                                                av[i][:],
                                                rbsb[:, i * 128:(i + 1) * 128],
                                                op=ALU.mult)
                    t2b = psT2.tile([128, 1024], F32, tag="t2")
                    for et2 in range(2):
                        for kt in range(2):
                            nc.tensor.matmul(
                                t2b[:, et2 * 128:(et2 + 1) * 128],
                                Wo_s[:, kt * 256 + et2 * 128: kt * 256 + (et2 + 1) * 128],
                                oT[:, kt * 128:(kt + 1) * 128],
                                start=(kt == 0), stop=(kt == 1))
                    fT = sbR.tile([128, 256], F32, tag="fT")
                    for et2 in range(2):
                        nc.scalar.activation(fT[:, et2 * 128:(et2 + 1) * 128],
                                             t2b[:, et2 * 128:(et2 + 1) * 128],
                                             AF.Identity,
                                             bias=boS_s[:, et2:et2 + 1])
                    t2c = psT2.tile([128, 1024], F32, tag="t2")
                    for et2 in range(2):
                        nc.tensor.transpose(t2c[:, et2 * 128:(et2 + 1) * 128],
                                            fT[:, et2 * 128:(et2 + 1) * 128],
                                            I128f_s[:])
                    fin = sbR.tile([128, 256], F32, tag="fin")
                    nc.vector.tensor_copy(
                        fin[:].rearrange("p (e c) -> p e c", e=2),
                        t2c[:, 0:256].rearrange("p (e c) -> p e c", e=2))
                    nc.sync.dma_start(t["out_o"][cg * 128:(cg + 1) * 128, :],
                                      fin[:])


def _host_prep(inputs):
    rec = np.asarray(inputs["rec_tokens"], np.float32)
    lig = np.asarray(inputs["lig_tokens"], np.float32)
    rc = np.asarray(inputs["rec_centers"], np.float32)
    lc = np.asarray(inputs["lig_centers"], np.float32)
    msk = np.asarray(inputs["lig_mask"])
    Wq = np.asarray(inputs["Wq"], np.float32)
    bq = np.asarray(inputs["bq"], np.float32)
    Wk = np.asarray(inputs["Wk"], np.float32)
    bk = np.asarray(inputs["bk"], np.float32)
    Wv = np.asarray(inputs["Wv"], np.float32)
    bv = np.asarray(inputs["bv"], np.float32)
    Wo = np.asarray(inputs["Wo"], np.float32)
    bo = np.asarray(inputs["bo"], np.float32)
    Wr = np.asarray(inputs["W_rbf"], np.float32)
    mus = np.asarray(inputs["mus"], np.float32)
    gamma = float(np.asarray(inputs["gamma"]))

    def pack_k(w):  # [256, M] -> [128, 2*M]
        return np.concatenate([w[0:128], w[128:256]], axis=1).copy()

    def perm128(w, b):  # head pair blocks at col/row 0 and 64
        wp = np.zeros((256, 512), np.float32)
        bp = np.zeros((128, 4), np.float32)
        for h in range(8):
            tt, j = h // 2, h % 2
            wp[:, tt * 128 + j * 64: tt * 128 + j * 64 + 32] = \
                w[:, h * 32:(h + 1) * 32]
            bp[j * 64:j * 64 + 32, tt] = b[h * 32:(h + 1) * 32]
        return wp, bp

    WqP, bqP = perm128(Wq * SCALE, bq * SCALE)
    WkP, bkP = perm128(Wk, bk)

    c1 = 2.0 * gamma * mus
    c0 = -gamma * mus * mus
    selz = np.zeros((128, 8 * 128), np.float32)
    for go in range(8):
        for l8 in range(8):
            for n in range(NR):
                m = l8 * 16 + n
                selz[go * 8 + l8, go * 128 + m] = c1[n]
                selz[64 + go * 8 + l8, go * 128 + m] = -gamma
    c0col = np.tile(c0, 8).reshape(128, 1).astype(np.float32)

    Wbd = np.zeros((128, 64), np.float32)
    for l8 in range(8):
        Wbd[l8 * 16:(l8 + 1) * 16, l8 * 8:(l8 + 1) * 8] = BIAS_SCALE * Wr

    I128f = np.eye(128, dtype=np.float32)
    repM = np.zeros((8, 256), np.float32)
    for i in range(2):
        for p in range(128):
            repM[4 * i + p // 32, i * 128 + p] = 1.0
    I128h = np.eye(128, dtype=ml_dtypes.bfloat16)

    maps = []
    for c in range(NCORES):
        b, q0 = c // 4, (c % 4) * RCORE
        x = rc[b, q0:q0 + RCORE]
        y = lc[b]
        featR = np.concatenate([(-2.0 * x).T, (x * x).sum(-1)[None, :],
                                np.ones((1, RCORE), np.float32)], 0)
        featL = np.concatenate([y.T, np.ones((1, TL), np.float32),
                                (y * y).sum(-1)[None, :]], 0)
        maps.append({
            "recT": pack_k(rec[b, q0:q0 + RCORE].T.copy()),
            "ligT": pack_k(lig[b].T.copy()),
            "WqP": pack_k(WqP), "WkP": pack_k(WkP),
            "Wv": pack_k(Wv), "Wo": pack_k(Wo),
            "bqP": bqP, "bkP": bkP,
            "bvB": np.broadcast_to(bv, (128, 256)).copy(),
            "boS": bo.reshape(2, 128).T.copy(),
            "featL": np.ascontiguousarray(featL, np.float32),
            "featR": np.ascontiguousarray(featR, np.float32),
            "Wbd": Wbd, "selz": selz, "c0col": c0col,
            "maskrow": (MASKNEG * msk[b].astype(np.float32)).reshape(1, TL),
            "I128f": I128f, "I128h": I128h, "repM": repM,
        })
    return maps


def kernel(**inputs):
    if "nc" not in _CACHE:
        _CACHE["nc"] = _build_module()
    nc = _CACHE["nc"]
    maps = _host_prep(inputs)
    res = run_bass_kernel_spmd(nc, maps, list(range(NCORES)))
    outs = res.results
    out = np.zeros((B, TR, D), np.float32)
    attn = np.zeros((B, H, TR, TL), np.float32)
    for c in range(NCORES):
        b, q0 = c // 4, (c % 4) * RCORE
        out[b, q0:q0 + RCORE] = outs[c]["out_o"]
        attn[b, :, q0:q0 + RCORE, :] = outs[c]["attn_o"]
    return out, attn


# revision 7
# speedup vs baseline: 1.3565x; 1.3565x over previous
"""Trainium2 Bass kernel for CrossAttentionWithDistBias.

Sharding: rows of (B=2, Tr=2048) split across 8 cores -> each core owns
one batch (c//4) and a 512-row block ((c%4)*512), computes all heads for
its rows (RBF bias work is shared across heads, so head-sharding would
duplicate the dominant exp cost 8x), and writes its slice of out/attn.
No collectives needed.

Device pipeline per core:
  D2^T via K=5 feature matmul; d = ACT sqrt; z[(l8,n),r] via selection
  matmul (puts the RBF index n on partitions); E = ACT exp (per-partition
  c0 bias); per-head bias via block-diagonal matmul; PE-transpose to
  [r,(l,h)]; QK matmul with K=33 (mask row folded in) + identity-matmul
  bias add into the same PSUM; ACT exp with fused row-sum (softmax denom);
  unnormalized attn @ v with the 1/rowsum folded into the small output;
  gpsimd normalizes attn for the HBM write.
"""

import numpy as np
import ml_dtypes

import concourse.bacc as bacc
import concourse.bass as bass
import concourse.mybir as mybir
import concourse.tile as tile
from concourse.bass_utils import run_bass_kernel_spmd

F32 = mybir.dt.float32
BF16 = mybir.dt.bfloat16
AF = mybir.ActivationFunctionType
ALU = mybir.AluOpType

B, TR, TL, D, H, DH, NR = 2, 2048, 1024, 256, 8, 32, 16
RCORE = 512
SCALE = DH ** -0.5
BIAS_SCALE = 2.0
MASKNEG = -30000.0
NCORES = 8

_CACHE: dict = {}


def _build_module():
    nc = bacc.Bacc("TRN2", target_bir_lowering=False, debug=False,
                   enable_asserts=False, num_devices=NCORES)

    def din(name, p, f, dt=F32):
        return nc.dram_tensor(name, [p, f], dt, kind="ExternalInput").ap()

    ins = dict(
        recT=din("recT", 128, 2 * RCORE),
        ligT=din("ligT", 128, 2 * TL),
        WqP=din("WqP", 128, 2 * 512),
        WkP=din("WkP", 128, 2 * 512),
        Wv=din("Wv", 128, 2 * 256),
        Wo=din("Wo", 128, 2 * 256),
        bqP=din("bqP", 128, 4),
        bkP=din("bkP", 128, 4),
        bvB=din("bvB", 128, 256),
        boS=din("boS", 128, 2),
        featL=din("featL", 5, TL),
        featR=din("featR", 5, RCORE),
        Wbd=din("Wbd", 128, 64),
        selz=din("selz", 128, 8 * 128),
        c0col=din("c0col", 128, 1),
        maskrow=din("maskrow", 1, TL),
        I128f=din("I128f", 128, 128),
        repM=din("repM", 8, 256),
        I128h=din("I128h", 128, 128, BF16),
    )
    ins["attn_o"] = nc.dram_tensor("attn_o", [H, RCORE, TL], F32,
                                   kind="ExternalOutput").ap()
    ins["out_o"] = nc.dram_tensor("out_o", [RCORE, D], F32,
                                  kind="ExternalOutput").ap()

    with tile.TileContext(nc) as tc:
        _body(nc, tc, ins)
    nc.compile()
    return nc


def _body(nc, tc, t):
    from contextlib import ExitStack
    es = ExitStack()
    with es:
        P = es.enter_context(tc.tile_pool(name="persist", bufs=1))
        recT_s = P.tile([128, 2 * RCORE], F32, tag="recT")
        ligT_s = P.tile([128, 2 * TL], F32, tag="ligT")
        WqP_s = P.tile([128, 2 * 512], F32, tag="WqP")
        WkP_s = P.tile([128, 2 * 512], F32, tag="WkP")
        Wv_s = P.tile([128, 2 * 256], F32, tag="Wv")
        Wo_s = P.tile([128, 2 * 256], F32, tag="Wo")
        bqP_s = P.tile([128, 4], F32, tag="bqP")
        bkP_s = P.tile([128, 4], F32, tag="bkP")
        bvB_s = P.tile([128, 256], F32, tag="bvB")
        boS_s = P.tile([128, 2], F32, tag="boS")
        featL_s = P.tile([5, TL], F32, tag="featL")
        featR_s = P.tile([5, RCORE], F32, tag="featR")
        Wbd_s = P.tile([128, 64], F32, tag="Wbd")
        selz_s = P.tile([128, 8 * 128], F32, tag="selz")
        c0_s = P.tile([128, 1], F32, tag="c0col")
        mrow_s = P.tile([1, TL], F32, tag="mask")
        I128f_s = P.tile([128, 128], F32, tag="idf")
        repM_s = P.tile([8, 256], F32, tag="repM")
        I128h_s = P.tile([128, 128], BF16, tag="idh")
        qTx_s = [P.tile([128, RCORE], F32, tag=f"qTx{i}", name=f"qTx{i}")
                 for i in range(4)]
        kTx_s = [P.tile([128, TL], F32, tag=f"kTx{i}", name=f"kTx{i}")
                 for i in range(4)]
        v_s = [P.tile([128, 256], F32, tag=f"v{i}", name=f"v{i}")
               for i in range(8)]
        rhsA_s = P.tile([128, 16 * RCORE], F32, tag="rhsA")
        brt_s = P.tile([128, 2 * 64 * 128], BF16, tag="brt")

        for name, dst in [("recT", recT_s), ("ligT", ligT_s), ("WqP", WqP_s),
                          ("WkP", WkP_s), ("Wv", Wv_s), ("Wo", Wo_s),
                          ("bqP", bqP_s), ("bkP", bkP_s), ("bvB", bvB_s),
                          ("boS", boS_s), ("featL", featL_s),
                          ("featR", featR_s), ("Wbd", Wbd_s),
                          ("selz", selz_s), ("c0col", c0_s),
                          ("maskrow", mrow_s), ("I128f", I128f_s),
                          ("repM", repM_s),
                          ("I128h", I128h_s)]:
            nc.sync.dma_start(dst[:], t[name])

        # ---------- stage A ----------
        with tc.tile_pool(name="psA", bufs=2, space="PSUM") as psA:
            for tt in range(4):
                ps = psA.tile([128, 1024], F32, tag="ps")
                for kt in range(2):
                    nc.tensor.matmul(
                        ps[:, 0:RCORE],
                        WqP_s[:, kt * 512 + tt * 128: kt * 512 + (tt + 1) * 128],
                        recT_s[:, kt * RCORE:(kt + 1) * RCORE],
                        start=(kt == 0), stop=(kt == 1))
                nc.scalar.activation(qTx_s[tt][:], ps[:, 0:RCORE],
                                     AF.Identity, bias=bqP_s[:, tt:tt + 1])
                nc.gpsimd.memset(qTx_s[tt][32:33, :], 1.0)
                nc.gpsimd.memset(qTx_s[tt][96:97, :], 1.0)
            for tt in range(4):
                ps = psA.tile([128, 1024], F32, tag="ps")
                for kt in range(2):
                    for nh in range(2):
                        nc.tensor.matmul(
                            ps[:, nh * 512:(nh + 1) * 512],
                            WkP_s[:, kt * 512 + tt * 128: kt * 512 + (tt + 1) * 128],
                            ligT_s[:, kt * TL + nh * 512: kt * TL + (nh + 1) * 512],
                            start=(kt == 0), stop=(kt == 1))
                nc.scalar.activation(kTx_s[tt][:], ps[:],
                                     AF.Identity, bias=bkP_s[:, tt:tt + 1])
                nc.sync.dma_start(kTx_s[tt][32:33, :], mrow_s[:])
                nc.sync.dma_start(kTx_s[tt][96:97, :], mrow_s[:])
            for c in range(8):
                ps = psA.tile([128, 1024], F32, tag="ps")
                for kt in range(2):
                    nc.tensor.matmul(
                        ps[:, 0:256],
                        ligT_s[:, kt * TL + c * 128: kt * TL + (c + 1) * 128],
                        Wv_s[:, kt * 256:(kt + 1) * 256],
                        start=(kt == 0), stop=(kt == 1))
                nc.vector.tensor_tensor(v_s[c][:], ps[:, 0:256], bvB_s[:],
                                        op=ALU.add)
            for c in range(8):
                ps = psA.tile([128, 1024], F32, tag="ps")
                nc.tensor.matmul(ps[:, 0:RCORE],
                                 featL_s[:, c * 128:(c + 1) * 128],
                                 featR_s[:], start=True, stop=True)
                for sub in range(2):
                    w = 2 * c + sub
                    nc.vector.tensor_scalar_max(
                        rhsA_s[64:128, w * RCORE:(w + 1) * RCORE],
                        ps[sub * 64:(sub + 1) * 64, 0:RCORE], 0.0)
                    nc.scalar.activation(
                        rhsA_s[0:64, w * RCORE:(w + 1) * RCORE],
                        rhsA_s[64:128, w * RCORE:(w + 1) * RCORE], AF.Sqrt)

        brt_v = brt_s[:].rearrange("p (rs gp c) -> p rs gp c", rs=2, gp=64)
        brt_mm = brt_s[:].rearrange("p (rs gp g2 l8 h) -> p rs gp g2 l8 h",
                                    rs=2, gp=64, g2=2, l8=8, h=8)

        for half in range(2):
            r0 = half * 256
            # ---------- stage B ----------
            with tc.tile_pool(name="psZ", bufs=2, space="PSUM") as psZ, \
                 tc.tile_pool(name="psB", bufs=1, space="PSUM") as psB, \
                 tc.tile_pool(name="psT", bufs=1, space="PSUM") as psT, \
                 tc.tile_pool(name="sbB", bufs=2) as sbB:
                bt = bsb = tt8 = None
                for quad in range(32):
                    zt = psZ.tile([128, 1024], F32, tag="z")
                    for gi in range(4):
                        g = quad * 4 + gi
                        w, go = g // 8, g % 8
                        nc.tensor.matmul(
                            zt[:, gi * 256:(gi + 1) * 256],
                            selz_s[:, go * 128:(go + 1) * 128],
                            rhsA_s[:, w * RCORE + r0: w * RCORE + r0 + 256],
                            start=True, stop=True)
                    et = sbB.tile([128, 1024], F32, tag="E")
                    nc.scalar.activation(et[:], zt[:], AF.Exp, bias=c0_s[:])
                    if quad % 2 == 0:
                        bt = psB.tile([128, 1024], F32, tag="bps")
                    for gi in range(4):
                        g = quad * 4 + gi
                        g2 = g % 2
                        gpl = ((quad % 2) * 4 + gi) // 2
                        nc.tensor.matmul(
                            bt[g2 * 64:(g2 + 1) * 64, gpl * 256:(gpl + 1) * 256],
                            Wbd_s[:], et[:, gi * 256:(gi + 1) * 256],
                            start=True, stop=True)
                    if quad % 2 == 1:
                        bsb = sbB.tile([128, 1024], F32, tag="bsb")
                        nc.vector.tensor_copy(bsb[:], bt[:])
                        tt8 = psT.tile([128, 1024], F32, tag="tps")
                        for gp in range(4):
                            for rsx in range(2):
                                nc.tensor.transpose(
                                    tt8[:, (gp * 2 + rsx) * 128:(gp * 2 + rsx + 1) * 128],
                                    bsb[:, gp * 256 + rsx * 128: gp * 256 + rsx * 128 + 128],
                                    I128f_s[:])
                        gp0 = (quad // 2) * 4
                        ov = tt8[:].rearrange("p (gp rsx c) -> p gp rsx c",
                                              gp=4, rsx=2)
                        nc.vector.tensor_copy(
                            brt_v[:, :, gp0:gp0 + 4, :].rearrange(
                                "p rsx gp c -> p gp rsx c"), ov)

            # ---------- stage C ----------
            with tc.tile_pool(name="psL", bufs=2, space="PSUM") as psL, \
                 tc.tile_pool(name="psT2", bufs=1, space="PSUM") as psT2, \
                 tc.tile_pool(name="psAV", bufs=2, space="PSUM") as psAV, \
                 tc.tile_pool(name="sbC", bufs=2) as sbC, \
                 tc.tile_pool(name="sbT", bufs=2) as sbT, \
                 tc.tile_pool(name="sbR", bufs=1) as sbR:
                rsum = sbR.tile([128, 16], F32, tag="rsum")
                rsi = sbR.tile([128, 16], F32, tag="rsi")
                av = [psAV.tile([128, 256], F32, tag="av", name=f"av{i}")
                      for i in range(2)]
                for h in range(8):
                    tt, j = h // 2, h % 2
                    atT = sbT.tile([128, 2048], F32, tag="atT")
                    for rs in range(2):
                        cg = half * 2 + rs
                        hc = rs * 8 + h
                        pl = psL.tile([128, 1024], F32, tag="L")
                        for nh in range(2):
                            nc.tensor.matmul(
                                pl[:, nh * 512:(nh + 1) * 512],
                                qTx_s[tt][64 * j:64 * j + 33,
                                          cg * 128:(cg + 1) * 128],
                                kTx_s[tt][64 * j:64 * j + 33,
                                          nh * 512:(nh + 1) * 512],
                                start=True, stop=False)
                        for nh in range(2):
                            nc.tensor.matmul(
                                pl[:, nh * 512:(nh + 1) * 512],
                                I128h_s[:],
                                brt_mm[:, rs, nh * 32:(nh + 1) * 32, :, :, h],
                                start=False, stop=True)
                        aun = sbC.tile([128, 1024], F32, tag="aun")
                        nc.scalar.activation(aun[:], pl[:], AF.Exp,
                                             accum_out=rsum[:, hc:hc + 1])
                        nc.vector.reciprocal(rsi[:, hc:hc + 1],
                                             rsum[:, hc:hc + 1])
                        t2 = psT2.tile([128, 1024], F32, tag="t2")
                        for lt in range(8):
                            nc.tensor.transpose(
                                t2[:, lt * 128:(lt + 1) * 128],
                                aun[:, lt * 128:(lt + 1) * 128], I128f_s[:])
                        nc.vector.tensor_copy(atT[:, rs * 1024:(rs + 1) * 1024],
                                              t2[:])
                        anrm = sbC.tile([128, 1024], F32, tag="anrm")
                        nc.vector.tensor_scalar_mul(anrm[:], aun[:],
                                                    rsi[:, hc:hc + 1])
                        nc.sync.dma_start(
                            t["attn_o"][h, cg * 128:(cg + 1) * 128, :], anrm[:])
                    atv = atT[:].rearrange("p (rs lt c) -> p rs lt c",
                                           rs=2, lt=8)
                    for lt in range(8):
                        nc.tensor.matmul(
                            av[h // 4][32 * (h % 4):32 * (h % 4) + 32, :],
                            v_s[lt][:, h * 32:(h + 1) * 32],
                            atv[:, :, lt, :],
                            start=(lt == 0), stop=(lt == 7),
                            tile_position=(0, 32 * (h % 4)))
                # out path (per chunk rs)
                for rs in range(2):
                    cg = half * 2 + rs
                    t2 = psT2.tile([128, 1024], F32, tag="t2")
                    nc.tensor.transpose(t2[0:8, 0:128],
                                        rsi[:, rs * 8:rs * 8 + 8], I128f_s[:])
                    rsiT = sbR.tile([8, 128], F32, tag="rsiT")
                    nc.vector.tensor_copy(rsiT[:], t2[0:8, 0:128])
                    rbps = psT2.tile([128, 1024], F32, tag="t2", name="rbps")
                    for i in range(2):
                        nc.tensor.matmul(rbps[:, i * 128:(i + 1) * 128],
                                         repM_s[:, i * 128:(i + 1) * 128],
                                         rsiT[:], start=True, stop=True)
                    rbsb = sbR.tile([128, 256], F32, tag="rbsb")
                    nc.vector.tensor_copy(rbsb[:], rbps[:, 0:256])
                    oT = sbR.tile([128, 256], F32, tag="oT")
                    for i in range(2):
                        nc.vector.tensor_tensor(
                            oT[:, i * 128:(i + 1) * 128],
                            av[i][:, rs * 128:(rs + 1) * 128],
                            rbsb[:, i * 128:(i + 1) * 128], op=ALU.mult)
                    t2b = psT2.tile([128, 1024], F32, tag="t2", name="t2b")
                    for et2 in range(2):
                        for kt in range(2):
                            nc.tensor.matmul(
                                t2b[:, et2 * 128:(et2 + 1) * 128],
                                Wo_s[:, kt * 256 + et2 * 128: kt * 256 + (et2 + 1) * 128],
                                oT[:, kt * 128:(kt + 1) * 128],
                                start=(kt == 0), stop=(kt == 1))
                    fT = sbR.tile([128, 256], F32, tag="fT")
                    for et2 in range(2):
                        nc.scalar.activation(fT[:, et2 * 128:(et2 + 1) * 128],
                                             t2b[:, et2 * 128:(et2 + 1) * 128],
                                             AF.Identity,
                                             bias=boS_s[:, et2:et2 + 1])
                    t2c = psT2.tile([128, 1024], F32, tag="t2", name="t2c")
                    for et2 in range(2):
                        nc.tensor.transpose(t2c[:, et2 * 128:(et2 + 1) * 128],
                                            fT[:, et2 * 128:(et2 + 1) * 128],
                                            I128f_s[:])
                    fin = sbR.tile([128, 256], F32, tag="fin")
                    nc.vector.tensor_copy(fin[:], t2c[:, 0:256])
                    nc.sync.dma_start(t["out_o"][cg * 128:(cg + 1) * 128, :],
                                      fin[:])


def _host_prep(inputs):
    rec = np.asarray(inputs["rec_tokens"], np.float32)
    lig = np.asarray(inputs["lig_tokens"], np.float32)
    rc = np.asarray(inputs["rec_centers"], np.float32)
    lc = np.asarray(inputs["lig_centers"], np.float32)
    msk = np.asarray(inputs["lig_mask"])
    Wq = np.asarray(inputs["Wq"], np.float32)
    bq = np.asarray(inputs["bq"], np.float32)
    Wk = np.asarray(inputs["Wk"], np.float32)
    bk = np.asarray(inputs["bk"], np.float32)
    Wv = np.asarray(inputs["Wv"], np.float32)
    bv = np.asarray(inputs["bv"], np.float32)
    Wo = np.asarray(inputs["Wo"], np.float32)
    bo = np.asarray(inputs["bo"], np.float32)
    Wr = np.asarray(inputs["W_rbf"], np.float32)
    mus = np.asarray(inputs["mus"], np.float32)
    gamma = float(np.asarray(inputs["gamma"]))

    def pack_k(w):  # [256, M] -> [128, 2*M]
        return np.concatenate([w[0:128], w[128:256]], axis=1).copy()

    def perm128(w, b):  # head pair blocks at col/row 0 and 64
        wp = np.zeros((256, 512), np.float32)
        bp = np.zeros((128, 4), np.float32)
        for h in range(8):
            tt, j = h // 2, h % 2
            wp[:, tt * 128 + j * 64: tt * 128 + j * 64 + 32] = \
                w[:, h * 32:(h + 1) * 32]
            bp[j * 64:j * 64 + 32, tt] = b[h * 32:(h + 1) * 32]
        return wp, bp

    WqP, bqP = perm128(Wq * SCALE, bq * SCALE)
    WkP, bkP = perm128(Wk, bk)

    c1 = 2.0 * gamma * mus
    c0 = -gamma * mus * mus
    selz = np.zeros((128, 8 * 128), np.float32)
    for go in range(8):
        for l8 in range(8):
            for n in range(NR):
                m = l8 * 16 + n
                selz[go * 8 + l8, go * 128 + m] = c1[n]
                selz[64 + go * 8 + l8, go * 128 + m] = -gamma
    c0col = np.tile(c0, 8).reshape(128, 1).astype(np.float32)

    Wbd = np.zeros((128, 64), np.float32)
    for l8 in range(8):
        Wbd[l8 * 16:(l8 + 1) * 16, l8 * 8:(l8 + 1) * 8] = BIAS_SCALE * Wr

    I128f = np.eye(128, dtype=np.float32)
    repM = np.zeros((8, 256), np.float32)
    for i in range(2):
        for p in range(128):
            repM[4 * i + p // 32, i * 128 + p] = 1.0
    I128h = np.eye(128, dtype=ml_dtypes.bfloat16)

    maps = []
    for c in range(NCORES):
        b, q0 = c // 4, (c % 4) * RCORE
        x = rc[b, q0:q0 + RCORE]
        y = lc[b]
        featR = np.concatenate([(-2.0 * x).T, (x * x).sum(-1)[None, :],
                                np.ones((1, RCORE), np.float32)], 0)
        featL = np.concatenate([y.T, np.ones((1, TL), np.float32),
                                (y * y).sum(-1)[None, :]], 0)
        maps.append({
            "recT": pack_k(rec[b, q0:q0 + RCORE].T.copy()),
            "ligT": pack_k(lig[b].T.copy()),
            "WqP": pack_k(WqP), "WkP": pack_k(WkP),
            "Wv": pack_k(Wv), "Wo": pack_k(Wo),
            "bqP": bqP, "bkP": bkP,
            "bvB": np.broadcast_to(bv, (128, 256)).copy(),
            "boS": bo.reshape(2, 128).T.copy(),
            "featL": np.ascontiguousarray(featL, np.float32),
            "featR": np.ascontiguousarray(featR, np.float32),
            "Wbd": Wbd, "selz": selz, "c0col": c0col,
            "maskrow": (MASKNEG * msk[b].astype(np.float32)).reshape(1, TL),
            "I128f": I128f, "I128h": I128h, "repM": repM,
        })
    return maps


def kernel(**inputs):
    if "nc" not in _CACHE:
        _CACHE["nc"] = _build_module()
    nc = _CACHE["nc"]
    maps = _host_prep(inputs)
    res = run_bass_kernel_spmd(nc, maps, list(range(NCORES)))
    outs = res.results
    out = np.zeros((B, TR, D), np.float32)
    attn = np.zeros((B, H, TR, TL), np.float32)
    for c in range(NCORES):
        b, q0 = c // 4, (c % 4) * RCORE
        out[b, q0:q0 + RCORE] = outs[c]["out_o"]
        attn[b, :, q0:q0 + RCORE, :] = outs[c]["attn_o"]
    return out, attn
